# revision 1
# baseline (speedup 1.0000x reference)
"""AttentionPredictorLSTM — data-parallel over 8 NeuronCores.

Strategy (per sharding hint): pure data parallel. Shard batch B=128 into
8 shards of 16, replicate all weights, run the full recurrent model on
each core, gather outputs. The sequential scans have no cross-batch
dependence, so no collectives are needed: we dispatch 8 independent
async executions (one per core) and stack the results.
"""
import numpy as np
import jax
import jax.numpy as jnp
from jax import lax
from functools import partial

F_SIZE = 128
N_HEADS = 8
HEAD_D = F_SIZE // N_HEADS
LANE_F = 64
NUM_PREDS = 6
N_CORES = 8
B_FULL = 128


def _layernorm(x, g, b, eps=1e-5):
    m = x.mean(-1, keepdims=True)
    v = jnp.var(x, axis=-1, keepdims=True)
    return (x - m) * lax.rsqrt(v + eps) * g + b


def _lstm_cell(x, h, c, wih, whh, bih, bhh):
    gates = x @ wih.T + bih + h @ whh.T + bhh
    i, f, g, o = jnp.split(gates, 4, axis=-1)
    c2 = jax.nn.sigmoid(f) * c + jax.nn.sigmoid(i) * jnp.tanh(g)
    h2 = jax.nn.sigmoid(o) * jnp.tanh(c2)
    return h2, c2


def _to_xy(dl, yaw):
    return jnp.concatenate([dl * jnp.cos(yaw), dl * jnp.sin(yaw)], axis=-1)


def _attention(tok_pos, h_ego, h, lane_tok, key_mask, wq, wk, wv, wo, pos_w, pos_b):
    B = h_ego.shape[0]
    tok = jnp.concatenate([h_ego, h], axis=1) + tok_pos @ pos_w + pos_b
    kv = jnp.concatenate([tok, lane_tok], axis=1)
    q = (tok @ wq).reshape(B, -1, N_HEADS, HEAD_D)
    k = (kv @ wk).reshape(B, -1, N_HEADS, HEAD_D)
    v = (kv @ wv).reshape(B, -1, N_HEADS, HEAD_D)
    s = jnp.einsum('bqhd,bkhd->bhqk', q, k) / jnp.asarray(np.sqrt(HEAD_D), jnp.float32)
    s = jnp.where(key_mask[:, None, None, :] > 0, s, -1e9)
    a = jax.nn.softmax(s, axis=-1)
    o = jnp.einsum('bhqk,bkhd->bqhd', a, v).reshape(B, -1, F_SIZE) @ wo
    tok = tok + o
    return tok[:, :1], tok[:, 1:]


@partial(jax.jit, static_argnames=('len_pred',))
def _forward(input, init_pos, lane_input, mask_input, lane_mask,
             conv_w, conv_b, pos_w, pos_b,
             lane_w1, lane_b1, lane_w2, lane_b2, lane_to_f,
             wq, wk, wv, wo, ln_ego_g, ln_ego_b, ln_g, ln_b,
             ego_wih, ego_whh, ego_bih, ego_bhh,
             veh_wih, veh_whh, veh_bih, veh_bhh,
             out_ego_w, out_ego_b, out_w, out_b, len_pred):
    T, B, N1, _ = input.shape
    V = N1 - 1
    Hh = T - 2

    x = jnp.transpose(input, (1, 3, 2, 0))
    feat = lax.conv_general_dilated(x, conv_w, (1, 1), 'VALID',
                                    dimension_numbers=('NCHW', 'OIHW', 'NCHW'))
    feat = feat + conv_b[None, :, None, None]
    feat = jnp.transpose(feat, (3, 0, 2, 1))
    h_ego_seq, h_seq = feat[:, :, :1], feat[:, :, 1:]

    lanes = jax.nn.relu(lane_input @ lane_w1 + lane_b1)
    lanes = jax.nn.relu(lanes @ lane_w2 + lane_b2).mean(axis=2)
    lane_tok = lanes @ lane_to_f
    key_mask = jnp.concatenate([mask_input, lane_mask], axis=1)

    pos0 = init_pos[0]
    pe0, pv0 = pos0[:, :1], pos0[:, 1:]

    def cell_step(hx_ego, cx_ego, hx, cx, he, hv, pe, pv):
        tok_pos = jnp.concatenate([pe, pv], axis=1)
        ae, av = _attention(tok_pos, he, hv, lane_tok, key_mask, wq, wk, wv, wo, pos_w, pos_b)
        ae = _layernorm(ae.reshape(B, F_SIZE), ln_ego_g, ln_ego_b)
        av = _layernorm(av.reshape(B * V, F_SIZE), ln_g, ln_b)
        hx_ego, cx_ego = _lstm_cell(ae, hx_ego, cx_ego, ego_wih, ego_whh, ego_bih, ego_bhh)
        hx, cx = _lstm_cell(av, hx, cx, veh_wih, veh_whh, veh_bih, veh_bhh)
        return hx_ego, cx_ego, hx, cx

    def hist_step(carry, xs):
        hx_ego, cx_ego, hx, cx, pe, pv = carry
        he, hv, xe, xv = xs
        pe = pe + _to_xy(xe[..., 0:1], xe[..., 1:2])
        pv = pv + _to_xy(xv[..., 0:1], xv[..., 1:2])
        hx_ego, cx_ego, hx, cx = cell_step(hx_ego, cx_ego, hx, cx, he, hv, pe, pv)
        return (hx_ego, cx_ego, hx, cx, pe, pv), None

    carry0 = (jnp.zeros((B, F_SIZE)), jnp.zeros((B, F_SIZE)),
              jnp.zeros((B * V, F_SIZE)), jnp.zeros((B * V, F_SIZE)), pe0, pv0)
    xs = (h_ego_seq, h_seq, input[:Hh, :, :1, :], input[:Hh, :, 1:, :])
    carry, _ = lax.scan(hist_step, carry0, xs)
    hx_ego, cx_ego, hx, cx, pe, pv = carry

    pe = jnp.repeat(pe[:, :, None, :], NUM_PREDS, axis=2)
    pv = jnp.repeat(pv[:, :, None, :], NUM_PREDS, axis=2)

    def pred_step(carry, _):
        hx_ego, cx_ego, hx, cx, pe, pv = carry
        he = hx_ego.reshape(B, 1, F_SIZE)
        hv = hx.reshape(B, V, F_SIZE)
        hx_ego, cx_ego, hx, cx = cell_step(hx_ego, cx_ego, hx, cx, he, hv,
                                           pe.mean(axis=2), pv.mean(axis=2))
        oe = (hx_ego @ out_ego_w + out_ego_b).reshape(B, 1, NUM_PREDS, 6)
        ov = (hx @ out_w + out_b).reshape(B, V, NUM_PREDS, 6)
        pe = pe + _to_xy(oe[..., 0:1], oe[..., 1:2])
        pv = pv + _to_xy(ov[..., 0:1], ov[..., 1:2])
        y = jnp.concatenate([
            jnp.concatenate([pe, oe[..., 2:6]], axis=-1),
            jnp.concatenate([pv, ov[..., 2:6]], axis=-1)], axis=1)
        return (hx_ego, cx_ego, hx, cx, pe, pv), y

    carry = (hx_ego, cx_ego, hx, cx, pe, pv)
    _, ys = lax.scan(pred_step, carry, None, length=len_pred)
    return ys


_SHARD_AXIS = {'input': 1, 'init_pos': 1, 'lane_input': 0,
               'mask_input': 0, 'lane_mask': 0}


def kernel(**inputs):
    len_pred = int(inputs.pop('len_pred'))
    devs = [d for d in jax.devices() if d.platform != 'cpu'][:N_CORES]
    if len(devs) < N_CORES:
        devs = jax.devices()[:N_CORES]
    bs = B_FULL // N_CORES

    # Build per-core input shards; weights replicated.
    names = list(inputs.keys())
    per_core = []
    for i in range(N_CORES):
        shard = {}
        for n in names:
            v = np.asarray(inputs[n])
            if n in _SHARD_AXIS:
                ax = _SHARD_AXIS[n]
                idx = [slice(None)] * v.ndim
                idx[ax] = slice(i * bs, (i + 1) * bs)
                shard[n] = v[tuple(idx)]
            else:
                shard[n] = v
        per_core.append(shard)

    # Dispatch asynchronously to all 8 cores, then gather.
    futs = []
    for i, dev in enumerate(devs):
        dshard = {n: jax.device_put(v, dev) for n, v in per_core[i].items()}
        futs.append(_forward(len_pred=len_pred, **dshard))
    outs = [np.asarray(f) for f in futs]
    return np.concatenate(outs, axis=1)  # [len_pred, B, N1, P, 6]
